# revision 1
# baseline (speedup 1.0000x reference)
"""Trainium2 Bass kernel for nn_DynamicPostionalBias.

Math: reference computes
    logits = einsum('nhid,hdj->nhij', query, rel_emb)        # [2,16,2048,4097]
    out[n,h,i,j] = logits[n,h,i, clip(j-i,-2047,2047)+2048]  # [2,16,2048,2048]
Since i,j in [0,2048), the clip is a no-op, so
    out[n,h,i,j] = sum_d q[n,h,i,d] * rel_emb[h,d, j-i+2048]
i.e. each output row i is a contiguous 2048-wide window of the logits row,
whose start shifts by -1 per row.

Strategy (8 NeuronCores): shard the 32 (n,h) pairs, 4 per core.  Host
pre-transposes q to [pair, d, i] and converts inputs to fp16 so no
on-device transpose is needed and DRAM traffic (the bottleneck for this
memory-regime problem) is halved; the output is stored as fp16 and
upcast on host (rel err ~5e-4, well under the 2e-2 gate).  Per pair and
per 128-row block, compute the needed logits window [128, 2175] via
fp16 matmuls ([64,128]^T @ [64,<=512] -> fp32 PSUM), evacuate PSUM with
two large converting copies (DVE: 1024 cols, ACT: 1024+128, the small
chunk alternating engines), then store with a single DMA whose SBUF
source access pattern walks the per-partition shifted windows: flat
logical index of row r is 127 + r*(W-1), i.e. ap=[[W-1, 128], [1, 2048]]
with offset 127 on a [128, W] tile.  All stores ride the SP HWDGE ring;
loads ride GpSimd SWDGE.  After tile scheduling, dedupe_ldweights()
strips back-to-back redundant PE weight reloads (the legalizer emits
one per matmul; all 5 chunk matmuls of a block share one stationary),
worth ~9us/core on hardware.
"""

import sys

import numpy as np

for _p in ("/opt/trn_rl_repo", "/root/.axon_site/_ro/trn_rl_repo"):
    if _p not in sys.path:
        sys.path.append(_p)

import concourse.bass as bass
import concourse.mybir as mybir
import concourse.tile as tile
from concourse import bacc
from concourse.ap import AP
from concourse.bass_utils import run_bass_kernel_spmd

B, H, S, D = 2, 16, 2048, 64
J = 2 * S + 1  # 4097
G = 4          # (n,h) pairs per core
NB = S // 128  # 16 row blocks
W = S + 128    # 2176 sbuf tile width (2175 computed; even alloc)
CW = S + 127   # 2175 computed window width
N_CORES = 8

# fp32r streams fp32 data through the PE at 1 cycle/row (vs 4 for fp32).
# float16 halves all DRAM traffic (the bottleneck): inputs are loaded as
# fp16 (PE computes fp16 x fp16 -> fp32 PSUM, products exact in fp32) and
# the output is stored as fp16 then upcast on host.  Total rel err ~1e-3,
# well inside the 2e-2 gate.
MM_DTYPE = mybir.dt.float16
OUT_DTYPE = mybir.dt.float16

F32 = mybir.dt.float32

# chunk layout of the 2176-wide window: 4 x 512 + 128 (PSUM bank = 512 fp32).
# Only 2175 columns are needed; the 2176th is harmless (E col cs+2175 <= 4096).
CHUNKS = [(0, 512), (512, 512), (1024, 512), (1536, 512), (2048, 128)]


def build_nc(mm_dtype=MM_DTYPE, nb=1, lp_bufs=4, reps=1, fold=True,
             dual_ring=False, swdge_loads=True, dve_chunks=(0, 2, 4),
             out_dtype=OUT_DTYPE, pool_chunks=(), scheme="bank2",
             alt_c=True, dedupe_lw=True, c_first=False,
             pair_interleave=False, c_share=1, split_load_rings=False,
             dedupe_w=True):
    """nb = row-blocks batched per output DMA (1 MiB each).

    E/qT for two consecutive pairs are folded into single 128-partition
    loads (pair gg in partitions 0-63, pair gg+1 in 64-127) for full DMA
    port coverage; matmuls address the upper half via base_partition=64.
    reps>1 re-runs the whole computation (for slope-based timing only).
    dual_ring alternates output stores between the SP and ACT HWDGE rings;
    swdge_loads issues input loads from GpSimd (SWDGE) instead of ACT.

    scheme="c512": one PSUM->SBUF copy per 512-col matmul chunk (5/block),
    split across DVE (dve_chunks) / Pool (pool_chunks) / ACT (rest).
    scheme="bank2": matmuls still 512-wide (PSUM bank limit) but into
    2-bank [128,1024] PSUM tiles; ONE DVE copy evacuates chunk A (1024),
    ACT evacuates B (1024) and C (128) — fewer, larger copies amortize
    the per-instruction overhead (~124-195ns).  alt_c alternates the C
    copy between ACT and DVE per block to balance the two engines.
    """
    nc = bacc.Bacc("TRN2", target_bir_lowering=False, debug=False)
    qT = nc.declare_dram_parameter("qT", [G, D, S], mm_dtype, isOutput=False)
    E = nc.declare_dram_parameter("E", [G, D, J], mm_dtype, isOutput=False)
    out = nc.declare_dram_parameter("out", [G, S, S], out_dtype, isOutput=True)

    from contextlib import ExitStack, nullcontext

    with tile.TileContext(nc) as tc, ExitStack() as stack:
        ep = stack.enter_context(tc.tile_pool(name="ep", bufs=2))
        qp = stack.enter_context(tc.tile_pool(name="qp", bufs=2))
        lp = stack.enter_context(tc.tile_pool(name="lp", bufs=lp_bufs))
        if scheme == "c512":
            pp = stack.enter_context(
                tc.tile_pool(name="pp", bufs=8, space=bass.MemorySpace.PSUM))
        else:
            # [128,1024] tiles A and B share one 3-buf rotation (6 banks);
            # C gets a 2-buf single-bank pool.  8 banks total.
            pa = stack.enter_context(
                tc.tile_pool(name="pa", bufs=3, space=bass.MemorySpace.PSUM))
            pc = stack.enter_context(
                tc.tile_pool(name="pc", bufs=2, space=bass.MemorySpace.PSUM))
        if True:
            with (tc.For_i(0, reps, 1) if reps > 1 else nullcontext()):
                step = 2 if fold else 1
                load_eng = nc.gpsimd if swdge_loads else nc.scalar
                for gg in range(0, G, step):
                    pd = step * D
                    # qT first (block 0 needs it); E split so the high
                    # half (cols >= 1792, used by blocks t=0,1) lands
                    # before the low half -- shortens the pipeline head.
                    qt = qp.tile([pd, S], mm_dtype, name=f"qt{gg}", tag="qt")
                    qsrc = qT[gg : gg + step].rearrange("g d s -> (g d) s")
                    et = ep.tile([pd, J], mm_dtype, name=f"et{gg}", tag="et")
                    esrc = E[gg : gg + step].rearrange("g d j -> (g d) j")
                    if gg == 0:
                        (nc.sync if split_load_rings else load_eng).dma_start(
                            qt[:], qsrc)
                        load_eng.dma_start(et[:, 1792:J], esrc[:, 1792:J])
                        load_eng.dma_start(et[:, 0:1792], esrc[:, 0:1792])
                    else:
                        load_eng.dma_start(qt[:], qsrc)
                        load_eng.dma_start(et[:], esrc)
                    if pair_interleave and step == 2:
                        gt_order = [(g, t0) for t0 in range(0, NB, nb)
                                    for g in range(gg, gg + step)]
                    else:
                        gt_order = [(g, t0) for g in range(gg, gg + step)
                                    for t0 in range(0, NB, nb)]
                    psC_tile = None
                    psC_ci = 0
                    for g, t0 in gt_order:
                        p0 = (g - gg) * D  # partition base within et/qt
                        if True:
                            lt = lp.tile([128, nb * W], out_dtype,
                                         name=f"lt{g}_{t0}", tag="lt")
                            for b in range(nb):
                                t = t0 + b
                                i0 = 128 * t
                                cs = S - i0 - 127  # window start column
                                qs = qt[p0 : p0 + D, i0 : i0 + 128]
                                lc = b * W

                                def mm(ps_slice, c0, w):
                                    nc.tensor.matmul(
                                        ps_slice,
                                        qs,
                                        et[p0 : p0 + D, cs + c0 : cs + c0 + w],
                                        start=True,
                                        stop=True,
                                    )

                                if scheme == "c512":
                                    for ci, (c0, w) in enumerate(CHUNKS):
                                        ps = pp.tile([128, 512], F32,
                                                     name=f"ps{g}_{t}_{ci}",
                                                     tag="ps")
                                        mm(ps[:, :w], c0, w)
                                        if ci in pool_chunks:
                                            eng = nc.gpsimd
                                        elif ci in dve_chunks:
                                            eng = nc.vector
                                        else:
                                            eng = nc.scalar
                                        if eng is nc.scalar:
                                            eng.copy(lt[:, lc + c0 : lc + c0 + w],
                                                     ps[:, :w])
                                        else:
                                            eng.tensor_copy(
                                                lt[:, lc + c0 : lc + c0 + w],
                                                ps[:, :w])
                                else:  # bank2
                                    psA = pa.tile([128, 1024], F32,
                                                  name=f"psA{g}_{t}", tag="ps2")
                                    psB = pa.tile([128, 1024], F32,
                                                  name=f"psB{g}_{t}", tag="ps2")
                                    # C chunks of c_share consecutive blocks
                                    # share one bank (4x128 fp32 fits 512):
                                    # avoids a per-block pool-buf wait, the
                                    # readiness inversion that made the
                                    # scheduler interleave adjacent blocks.
                                    if psC_tile is None or psC_ci >= c_share:
                                        psC_tile = pc.tile(
                                            [128, 512], F32,
                                            name=f"psC{g}_{t}", tag="psC")
                                        psC_ci = 0
                                    c0_sl = 128 * psC_ci
                                    psC_ci += 1
                                    csl = psC_tile[:, c0_sl : c0_sl + 128]
                                    if c_first:
                                        mm(csl, 2048, 128)
                                    mm(psA[:, 0:512], 0, 512)
                                    mm(psA[:, 512:1024], 512, 512)
                                    mm(psB[:, 0:512], 1024, 512)
                                    mm(psB[:, 512:1024], 1536, 512)
                                    if not c_first:
                                        mm(csl, 2048, 128)
                                    nc.vector.tensor_copy(
                                        lt[:, lc : lc + 1024], psA[:]
                                    )
                                    nc.scalar.copy(
                                        lt[:, lc + 1024 : lc + 2048],
                                        psB[:]
                                    )
                                    if alt_c and (t % 2 == 1):
                                        nc.vector.tensor_copy(
                                            lt[:, lc + 2048 : lc + 2176],
                                            csl)
                                    else:
                                        nc.scalar.copy(
                                            lt[:, lc + 2048 : lc + 2176],
                                            csl)
                            # out[g, 128*(t0+b)+r, j] = lt[r, b*W + 127-r + j]
                            if nb == 1:
                                src_ap = [[W - 1, 128], [1, S]]
                            else:
                                src_ap = [[W, nb], [nb * W - 1, 128], [1, S]]
                            src = AP(
                                tensor=lt.tensor,
                                offset=lt.offset + 127,
                                ap=src_ap,
                            )
                            st_eng = (
                                nc.scalar
                                if dual_ring and (t0 % 2 == 1)
                                else nc.sync
                            )
                            st_eng.dma_start(
                                out[g, 128 * t0 : 128 * (t0 + nb), :], src
                            )
    if dedupe_lw:
        dedupe_ldweights(nc)
    if dedupe_w:
        dedupe_waits(nc)
    nc.compile()
    return nc


def dedupe_ldweights(nc):
    """Remove back-to-back InstLdweights that reload the identical
    stationary (the legalizer emits one per matmul; all 5 chunk matmuls
    of a row block share one stationary).  The PE queue is in-order, so
    the weights stay resident across the following matmuls.  Sem waits /
    updates of a deleted load are transferred to its paired matmul
    (which executes in the same queue slot) so semaphore thresholds are
    preserved.  Runs pre-compile: generate_event_semaphores afterwards
    re-normalizes multi-wait instructions.
    """
    removed = 0
    last_sig = {}
    pending_sync = None
    for blk in nc.m.functions[0].blocks:
        if blk.IsLoopEntry or blk.IsPredicated:
            last_sig = {}
        insts = list(blk.instructions)
        new_insts = []
        for inst in insts:
            if isinstance(inst, mybir.InstLdweights):
                ap = inst.ins[0]
                # PE row-slot: folded pairs live at partitions 0-63 /
                # 64-127 (tile_position row 0 / 64), whose stationaries
                # are independent in the PE array.
                slot = ap.offset // (64 * S)
                sig = repr(ap)
                if sig == last_sig.get(slot):
                    si = inst.sync_info
                    if si is not None and (len(si.on_wait) or len(si.on_update)):
                        if pending_sync is None:
                            pending_sync = ([], [])
                        pending_sync[0].extend(si.on_wait)
                        pending_sync[1].extend(si.on_update)
                    removed += 1
                    continue
                last_sig[slot] = sig
            elif isinstance(inst, mybir.InstMatmult):
                if pending_sync is not None:
                    si = inst.sync_info
                    if si is None:
                        inst.sync_info = mybir.SyncInfo(
                            on_wait=pending_sync[0], on_update=pending_sync[1]
                        )
                    else:
                        si.on_wait = list(si.on_wait) + pending_sync[0]
                        si.on_update = list(si.on_update) + pending_sync[1]
                    pending_sync = None
            new_insts.append(inst)
        if len(new_insts) != len(insts):
            blk.instructions = new_insts
    assert pending_sync is None, "dangling sync from deleted ldweights"
    return removed


_WAIT_ENGINES = ("EngineType.PE", "EngineType.DVE",
                 "EngineType.Activation", "EngineType.SP")


def dedupe_waits(nc):
    """Drop semaphore waits that are subsumed by an earlier wait in the
    same engine's in-order queue (same sem, earlier threshold >= later,
    'sem-ge-imm' mode only — these are monotonically-incrementing
    completion counters).  The later instruction can then only issue
    earlier in wall-clock, never before its guaranteed dependencies.
    Fewer waits also means fewer InstEventSemaphore helpers after
    generate_event_semaphores (each costs SEQ time on the engine).
    Tracking resets at loop entries, barriers/drains/ISA ops, and on any
    non-increment update to a sem.
    """
    seen = {e: {} for e in _WAIT_ENGINES}
    removed = 0

    def clear_all():
        for e in _WAIT_ENGINES:
            seen[e].clear()

    for blk in nc.m.functions[0].blocks:
        if blk.IsLoopEntry or blk.IsPredicated:
            clear_all()
        for inst in blk.instructions:
            if isinstance(inst, (mybir.InstDrain, mybir.InstISA,
                                 mybir.InstAllEngineBarrier)):
                clear_all()
                continue
            eng = str(getattr(inst, "engine", ""))
            si = inst.sync_info
            if si is not None:
                for upd in si.on_update:
                    # non-increment update (set/clear) invalidates history
                    mode = getattr(upd, "update_mode", "")
                    if "inc" not in str(mode):
                        for e in _WAIT_ENGINES:
                            seen[e].pop(getattr(upd, "id", None), None)
            if eng not in _WAIT_ENGINES or si is None or not si.on_wait:
                continue
            tbl = seen[eng]
            keep = []
            for wt in si.on_wait:
                if wt.wait_mode == "sem-ge-imm" and wt.wait_reg is None:
                    prev = tbl.get(wt.id)
                    if prev is not None and prev >= wt.wait_value:
                        removed += 1
                        continue
                    tbl[wt.id] = max(prev or 0, wt.wait_value)
                keep.append(wt)
            if len(keep) != len(si.on_wait):
                si.on_wait = keep
    return removed


_NC_CACHE = {}

NB_PER_DMA = 1
LP_BUFS = 4


def _np_dtype(mm_dtype):
    return np.float16 if mm_dtype == mybir.dt.float16 else np.float32


def _get_nc(mm_dtype=MM_DTYPE, nb=None, lp_bufs=None, **kw):
    nb = NB_PER_DMA if nb is None else nb
    lp_bufs = LP_BUFS if lp_bufs is None else lp_bufs
    key = (str(mm_dtype), nb, lp_bufs, tuple(sorted(kw.items())))
    if key not in _NC_CACHE:
        _NC_CACHE[key] = build_nc(mm_dtype, nb=nb, lp_bufs=lp_bufs, **kw)
    return _NC_CACHE[key]


def make_in_maps(query, rel_emb, mm_dtype=MM_DTYPE):
    npdt = _np_dtype(mm_dtype)
    query = np.asarray(query, dtype=np.float32).astype(npdt)
    rel_emb = np.asarray(rel_emb, dtype=np.float32).astype(npdt)
    # [B,H,S,D] -> [32, D, S], pair p = n*16 + h
    qTt = np.ascontiguousarray(
        query.reshape(B * H, S, D).transpose(0, 2, 1)
    )
    in_maps = []
    for k in range(N_CORES):
        h0 = 4 * (k % 4)
        in_maps.append(
            {
                "qT": qTt[4 * k : 4 * k + 4],
                "E": np.ascontiguousarray(rel_emb[h0 : h0 + 4]),
            }
        )
    return in_maps


def run_sharded(query, rel_emb, trace=False, mm_dtype=MM_DTYPE, **kw):
    nc = _get_nc(mm_dtype, **kw)
    in_maps = make_in_maps(query, rel_emb, mm_dtype)
    last_exc = None
    for attempt in range(3):
        if attempt:
            # transient device errors (e.g. NRT_EXEC_UNIT_UNRECOVERABLE)
            # have been observed to clear after a short cooldown
            import time

            time.sleep(20 * attempt)
        try:
            res = run_bass_kernel_spmd(
                nc, in_maps, list(range(N_CORES)), trace=trace
            )
            break
        except Exception as exc:  # noqa: BLE001 - retry transient device faults
            last_exc = exc
    else:
        raise last_exc
    full = np.empty((B * H, S, S), dtype=np.float32)
    for k in range(N_CORES):
        full[4 * k : 4 * k + 4] = res.results[k]["out"]  # upcasts if fp16
    return full.reshape(B, H, S, S), res


def kernel(query, rel_emb, sequence_length=None):
    out, _ = run_sharded(query, rel_emb, trace=False)
    return out


# ---------------------------------------------------------------------------
# Timing harness (dev only): re-runnable sharded executable without donation,
# pipelined dispatch, null-kernel baseline subtraction.
# ---------------------------------------------------------------------------


def _prepare_exec(nc, in_maps, chain=1):
    import jax
    from jax.experimental.shard_map import shard_map
    from jax.sharding import Mesh, NamedSharding, PartitionSpec

    from concourse import bass2jax, mybir as mb

    bass2jax.install_neuronx_cc_hook()
    n_cores = len(in_maps)

    in_names, out_names, out_avals, zero_outs = [], [], [], []
    for alloc in nc.m.functions[0].allocations:
        if not isinstance(alloc, mb.MemoryLocationSet):
            continue
        name = alloc.memorylocations[0].name
        if alloc.kind == "ExternalInput":
            in_names.append(name)
        elif alloc.kind == "ExternalOutput":
            out_names.append(name)
            shape = tuple(alloc.tensor_shape)
            dtype = mb.dt.np(alloc.dtype)
            out_avals.append(jax.core.ShapedArray(shape, dtype))
            zero_outs.append(np.zeros(shape, dtype))
    partition_name = (
        nc.partition_id_tensor.name if nc.partition_id_tensor else None
    )
    if partition_name is not None and partition_name in in_names:
        in_names.remove(partition_name)
    n_params = len(in_names)
    in_names = in_names + out_names
    if partition_name is not None:
        in_names.append(partition_name)

    def _body(*args):
        operands = list(args)
        if partition_name is not None:
            operands.append(bass2jax.partition_id_tensor())
        for _ in range(chain):
            outs = bass2jax._bass_exec_p.bind(
                *operands,
                out_avals=tuple(out_avals),
                in_names=tuple(in_names),
                out_names=tuple(out_names),
                lowering_input_output_aliases=(),
                sim_require_finite=True,
                sim_require_nnan=True,
                nc=nc,
            )
        return tuple(outs)

    devices = jax.devices()[:n_cores]
    mesh = Mesh(np.asarray(devices), ("core",))
    spec = PartitionSpec("core")
    sharded = jax.jit(
        shard_map(
            _body,
            mesh=mesh,
            in_specs=(spec,) * (n_params + len(out_names)),
            out_specs=(spec,) * len(out_names),
            check_rep=False,
        ),
        keep_unused=True,
    )
    sh = NamedSharding(mesh, spec)
    per_core = [[np.asarray(m[name]) for name in in_names[:n_params]]
                for m in in_maps]
    args = [
        jax.device_put(
            np.concatenate([per_core[c][i] for c in range(n_cores)], axis=0), sh
        )
        for i in range(n_params)
    ]
    args += [
        jax.device_put(
            np.zeros((n_cores * z.shape[0], *z.shape[1:]), z.dtype), sh
        )
        for z in zero_outs
    ]
    return sharded, args


def build_null_nc(mm_dtype=MM_DTYPE, out_dtype=OUT_DTYPE):
    """Same I/O signature, near-zero work: for dispatch-overhead baseline."""
    nc = bacc.Bacc("TRN2", target_bir_lowering=False, debug=False)
    qT = nc.declare_dram_parameter("qT", [G, D, S], mm_dtype, isOutput=False)
    nc.declare_dram_parameter("E", [G, D, J], mm_dtype, isOutput=False)
    out = nc.declare_dram_parameter("out", [G, S, S], out_dtype, isOutput=True)
    with tile.TileContext(nc) as tc:
        with tc.tile_pool(name="p", bufs=1) as p:
            t = p.tile([64, 128], mm_dtype, name="t")
            t2 = p.tile([64, 128], out_dtype, name="t2")
            nc.sync.dma_start(t[:], qT[0, :, :128])
            nc.vector.tensor_copy(t2[:], t[:])
            nc.sync.dma_start(out[0, :64, :128], t2[:])
    nc.compile()
    return nc


def _time_callable(f, args, iters, reps=3):
    import time as _t

    import jax

    out = f(*args)
    jax.block_until_ready(out)
    best = float("inf")
    for _ in range(reps):
        t0 = _t.perf_counter()
        outs = [f(*args) for _ in range(iters)]
        jax.block_until_ready(outs)
        t1 = _t.perf_counter()
        best = min(best, (t1 - t0) / iters)
        del outs
    return best


def model_time_ns(mm_dtype=MM_DTYPE, **kw):
    """Instruction-level cost-model (TimelineSim) estimate for one core."""
    from concourse.timeline_sim import TimelineSim

    return TimelineSim(_get_nc(mm_dtype, **kw), trace=False).simulate()


def time_kernel(query, rel_emb, iters=6, mm_dtype=MM_DTYPE, rounds=4, **kw):
    """Differential wall-clock: alternate (kernel, null-kernel with same I/O)
    pipelined batches; report median of per-round differences.  The axon
    dispatch overhead (~3 ms/call, noisy) mostly cancels; the cost-model
    estimate is typically the more trustworthy number."""
    in_maps = make_in_maps(query, rel_emb, mm_dtype)
    f, args = _prepare_exec(_get_nc(mm_dtype, **kw), in_maps)
    f0, args0 = _prepare_exec(build_null_nc(mm_dtype), in_maps)
    tks, tns = [], []
    for _ in range(rounds):
        tks.append(_time_callable(f, args, iters, reps=1))
        tns.append(_time_callable(f0, args0, iters, reps=1))
    best = min(tks) - min(tns)
    print(f"  min kernel={min(tks)*1e6:.0f}us  min null={min(tns)*1e6:.0f}us  "
          f"diff-of-mins={best*1e6:.0f}us")
    return best * 1e9



# revision 24
# speedup vs baseline: 1.1806x; 1.1806x over previous
"""Trainium2 Bass kernel for nn_DynamicPostionalBias.

Math: reference computes
    logits = einsum('nhid,hdj->nhij', query, rel_emb)        # [2,16,2048,4097]
    out[n,h,i,j] = logits[n,h,i, clip(j-i,-2047,2047)+2048]  # [2,16,2048,2048]
Since i,j in [0,2048), the clip is a no-op, so
    out[n,h,i,j] = sum_d q[n,h,i,d] * rel_emb[h,d, j-i+2048]
i.e. each output row i is a contiguous 2048-wide window of the logits row,
whose start shifts by -1 per row.

Strategy (8 NeuronCores): shard the 32 (n,h) pairs, 4 per core.  Host
pre-transposes q to [pair, d, i] and converts inputs to fp16 so no
on-device transpose is needed and DRAM traffic (the bottleneck for this
memory-regime problem) is halved; the output is stored as fp16 and
upcast on host (rel err ~5e-4, well under the 2e-2 gate).  Per pair and
per 128-row block, compute the needed logits window [128, 2175] via
fp16 matmuls ([64,128]^T @ [64,<=512] -> fp32 PSUM), evacuate PSUM with
two large converting copies (DVE: 1024 cols, ACT: 1024+128, the small
chunk alternating engines), then store with a single DMA whose SBUF
source access pattern walks the per-partition shifted windows: flat
logical index of row r is 127 + r*(W-1), i.e. ap=[[W-1, 128], [1, 2048]]
with offset 127 on a [128, W] tile.  All stores ride the SP HWDGE ring;
loads ride GpSimd SWDGE.  After tile scheduling, dedupe_ldweights()
strips back-to-back redundant PE weight reloads (the legalizer emits
one per matmul; all 5 chunk matmuls of a block share one stationary),
worth ~9us/core on hardware.
"""

import sys

import numpy as np

for _p in ("/opt/trn_rl_repo", "/root/.axon_site/_ro/trn_rl_repo"):
    if _p not in sys.path:
        sys.path.append(_p)

import concourse.bass as bass
import concourse.mybir as mybir
import concourse.tile as tile
from concourse import bacc
from concourse.ap import AP
from concourse.bass_utils import run_bass_kernel_spmd

B, H, S, D = 2, 16, 2048, 64
J = 2 * S + 1  # 4097
G = 4          # (n,h) pairs per core
NB = S // 128  # 16 row blocks
W = S + 128    # 2176 sbuf tile width (2175 computed; even alloc)
CW = S + 127   # 2175 computed window width
N_CORES = 8

# fp32r streams fp32 data through the PE at 1 cycle/row (vs 4 for fp32).
# float16 halves all DRAM traffic (the bottleneck): inputs are loaded as
# fp16 (PE computes fp16 x fp16 -> fp32 PSUM, products exact in fp32) and
# the output is stored as fp16 then upcast on host.  Total rel err ~1e-3,
# well inside the 2e-2 gate.
MM_DTYPE = mybir.dt.float16
OUT_DTYPE = mybir.dt.float16

F32 = mybir.dt.float32

# chunk layout of the 2176-wide window: 4 x 512 + 128 (PSUM bank = 512 fp32).
# Only 2175 columns are needed; the 2176th is harmless (E col cs+2175 <= 4096).
CHUNKS = [(0, 512), (512, 512), (1024, 512), (1536, 512), (2048, 128)]


def build_nc(mm_dtype=MM_DTYPE, nb=1, lp_bufs=4, reps=1, fold=True,
             dual_ring=False, swdge_loads=True, dve_chunks=(0, 2, 4),
             out_dtype=OUT_DTYPE, pool_chunks=(), scheme="bank2",
             alt_c=True, dedupe_lw=True, c_first=False,
             pair_interleave=False, c_share=1, split_load_rings=False,
             dedupe_w=True, stores=True, eshare=False, copies=True,
             matmuls=True, lw_mode="sig", fine_loads=False):
    """nb = row-blocks batched per output DMA (1 MiB each).

    E/qT for two consecutive pairs are folded into single 128-partition
    loads (pair gg in partitions 0-63, pair gg+1 in 64-127) for full DMA
    port coverage; matmuls address the upper half via base_partition=64.
    reps>1 re-runs the whole computation (for slope-based timing only).
    dual_ring alternates output stores between the SP and ACT HWDGE rings;
    swdge_loads issues input loads from GpSimd (SWDGE) instead of ACT.

    scheme="c512": one PSUM->SBUF copy per 512-col matmul chunk (5/block),
    split across DVE (dve_chunks) / Pool (pool_chunks) / ACT (rest).
    scheme="bank2": matmuls still 512-wide (PSUM bank limit) but into
    2-bank [128,1024] PSUM tiles; ONE DVE copy evacuates chunk A (1024),
    ACT evacuates B (1024) and C (128) — fewer, larger copies amortize
    the per-instruction overhead (~124-195ns).  alt_c alternates the C
    copy between ACT and DVE per block to balance the two engines.
    """
    nc = bacc.Bacc("TRN2", target_bir_lowering=False, debug=False)
    qT = nc.declare_dram_parameter("qT", [G, D, S], mm_dtype, isOutput=False)
    # eshare: pairs are ordered (n0,h0),(n0,h1),(n1,h0),(n1,h1) so a single
    # folded E load [128, J] (h0 in partitions 0-63, h1 in 64-127) serves
    # all four pairs -- halves E DRAM traffic (1.05 MB/core saved).
    E = nc.declare_dram_parameter(
        "E", [2 if eshare else G, D, J], mm_dtype, isOutput=False)
    out = nc.declare_dram_parameter("out", [G, S, S], out_dtype, isOutput=True)

    from contextlib import ExitStack, nullcontext

    with tile.TileContext(nc) as tc, ExitStack() as stack:
        ep = stack.enter_context(tc.tile_pool(name="ep", bufs=2))
        qp = stack.enter_context(tc.tile_pool(name="qp", bufs=2))
        lp = stack.enter_context(tc.tile_pool(name="lp", bufs=lp_bufs))
        if scheme == "c512":
            pp = stack.enter_context(
                tc.tile_pool(name="pp", bufs=8, space=bass.MemorySpace.PSUM))
        else:
            # [128,1024] tiles A and B share one 3-buf rotation (6 banks);
            # C gets a 2-buf single-bank pool.  8 banks total.
            pa = stack.enter_context(
                tc.tile_pool(name="pa", bufs=3, space=bass.MemorySpace.PSUM))
            pc = stack.enter_context(
                tc.tile_pool(name="pc", bufs=2, space=bass.MemorySpace.PSUM))
        if True:
            with (tc.For_i(0, reps, 1) if reps > 1 else nullcontext()):
                step = 2 if fold else 1
                load_eng = nc.gpsimd if swdge_loads else nc.scalar
                et_shared = None
                for gg in range(0, G, step):
                    pd = step * D
                    # qT first (block 0 needs it); E split so the high
                    # half (cols >= 1792, used by blocks t=0,1) lands
                    # before the low half -- shortens the pipeline head.
                    qt = qp.tile([pd, S], mm_dtype, name=f"qt{gg}", tag="qt")
                    qsrc = qT[gg : gg + step].rearrange("g d s -> (g d) s")
                    if eshare:
                        if et_shared is None:
                            et = ep.tile([pd, J], mm_dtype, name="et", tag="et")
                            esrc = E[0:2].rearrange("g d j -> (g d) j")
                            if fine_loads:
                                # q on the SP HWDGE ring, E on SWDGE --
                                # parallel rings; small q prefix + high-E
                                # first so block 0 starts ~1.8us in.
                                nc.sync.dma_start(qt[:, 0:256], qsrc[:, 0:256])
                                load_eng.dma_start(et[:, 1665:J],
                                                   esrc[:, 1665:J])
                                nc.sync.dma_start(qt[:, 256:S],
                                                  qsrc[:, 256:S])
                                load_eng.dma_start(et[:, 0:1665],
                                                   esrc[:, 0:1665])
                            else:
                                (nc.sync if split_load_rings
                                 else load_eng).dma_start(qt[:], qsrc)
                                load_eng.dma_start(et[:, 1792:J],
                                                   esrc[:, 1792:J])
                                load_eng.dma_start(et[:, 0:1792],
                                                   esrc[:, 0:1792])
                            et_shared = et
                        else:
                            et = et_shared
                            load_eng.dma_start(qt[:], qsrc)
                    else:
                        et = ep.tile([pd, J], mm_dtype, name=f"et{gg}", tag="et")
                        esrc = E[gg : gg + step].rearrange("g d j -> (g d) j")
                        if gg == 0:
                            (nc.sync if split_load_rings
                             else load_eng).dma_start(qt[:], qsrc)
                            load_eng.dma_start(et[:, 1792:J], esrc[:, 1792:J])
                            load_eng.dma_start(et[:, 0:1792], esrc[:, 0:1792])
                        else:
                            load_eng.dma_start(qt[:], qsrc)
                            load_eng.dma_start(et[:], esrc)
                    if pair_interleave and step == 2:
                        gt_order = [(g, t0) for t0 in range(0, NB, nb)
                                    for g in range(gg, gg + step)]
                    else:
                        gt_order = [(g, t0) for g in range(gg, gg + step)
                                    for t0 in range(0, NB, nb)]
                    psC_tile = None
                    psC_ci = 0
                    for g, t0 in gt_order:
                        p0 = (g - gg) * D  # partition base within et/qt
                        if True:
                            lt = lp.tile([128, nb * W], out_dtype,
                                         name=f"lt{g}_{t0}", tag="lt")
                            for b in range(nb):
                                t = t0 + b
                                i0 = 128 * t
                                cs = S - i0 - 127  # window start column
                                qs = qt[p0 : p0 + D, i0 : i0 + 128]
                                lc = b * W

                                def mm(ps_slice, c0, w):
                                    if not matmuls:
                                        return
                                    nc.tensor.matmul(
                                        ps_slice,
                                        qs,
                                        et[p0 : p0 + D, cs + c0 : cs + c0 + w],
                                        start=True,
                                        stop=True,
                                    )

                                if scheme == "c512":
                                    chunk_iter = list(enumerate(CHUNKS))
                                    if c_first:
                                        chunk_iter = (chunk_iter[-1:]
                                                      + chunk_iter[:-1])
                                    dvec = dve_chunks
                                    if dvec == "alt":
                                        # C chunk engine alternates per block
                                        dvec = ((0, 2, 4) if t % 2 == 0
                                                else (0, 2))
                                    for ci, (c0, w) in chunk_iter:
                                        ps = pp.tile([128, 512], F32,
                                                     name=f"ps{g}_{t}_{ci}",
                                                     tag="ps")
                                        mm(ps[:, :w], c0, w)
                                        if ci in pool_chunks:
                                            eng = nc.gpsimd
                                        elif ci in dvec:
                                            eng = nc.vector
                                        else:
                                            eng = nc.scalar
                                        if eng is nc.scalar:
                                            eng.copy(lt[:, lc + c0 : lc + c0 + w],
                                                     ps[:, :w])
                                        else:
                                            eng.tensor_copy(
                                                lt[:, lc + c0 : lc + c0 + w],
                                                ps[:, :w])
                                else:  # bank2
                                    psA = pa.tile([128, 1024], F32,
                                                  name=f"psA{g}_{t}", tag="ps2")
                                    psB = pa.tile([128, 1024], F32,
                                                  name=f"psB{g}_{t}", tag="ps2")
                                    # C chunks of c_share consecutive blocks
                                    # share one bank (4x128 fp32 fits 512):
                                    # avoids a per-block pool-buf wait, the
                                    # readiness inversion that made the
                                    # scheduler interleave adjacent blocks.
                                    if psC_tile is None or psC_ci >= c_share:
                                        psC_tile = pc.tile(
                                            [128, 512], F32,
                                            name=f"psC{g}_{t}", tag="psC")
                                        psC_ci = 0
                                    c0_sl = 128 * psC_ci
                                    psC_ci += 1
                                    csl = psC_tile[:, c0_sl : c0_sl + 128]
                                    if c_first:
                                        mm(csl, 2048, 128)
                                    mm(psA[:, 0:512], 0, 512)
                                    mm(psA[:, 512:1024], 512, 512)
                                    mm(psB[:, 0:512], 1024, 512)
                                    mm(psB[:, 512:1024], 1536, 512)
                                    if not c_first:
                                        mm(csl, 2048, 128)
                                    if not copies:
                                        pass
                                    elif scheme == "b2s":
                                        # bank2 PSUM tiles; DVE one 1024
                                        # copy, ACT two 512 copies (finer
                                        # release downstream), C alternates.
                                        nc.vector.tensor_copy(
                                            lt[:, lc : lc + 1024], psA[:])
                                        nc.scalar.copy(
                                            lt[:, lc + 1024 : lc + 1536],
                                            psB[:, 0:512])
                                        nc.scalar.copy(
                                            lt[:, lc + 1536 : lc + 2048],
                                            psB[:, 512:1024])
                                        if alt_c and (t % 2 == 1):
                                            nc.vector.tensor_copy(
                                                lt[:, lc + 2048 : lc + 2176],
                                                csl)
                                        else:
                                            nc.scalar.copy(
                                                lt[:, lc + 2048 : lc + 2176],
                                                csl)
                                    elif scheme == "bal3":
                                        # 3-engine evacuation: DVE 768,
                                        # ACT 256+512, Pool 512+128 --
                                        # balances the per-engine busy legs
                                        # (DVE runs at 0.96 GHz vs 1.2 for
                                        # ACT/Pool) and pulls the idle Pool
                                        # engine into the rotation.
                                        nc.vector.tensor_copy(
                                            lt[:, lc : lc + 768],
                                            psA[:, 0:768])
                                        nc.scalar.copy(
                                            lt[:, lc + 768 : lc + 1024],
                                            psA[:, 768:1024])
                                        nc.scalar.copy(
                                            lt[:, lc + 1024 : lc + 1536],
                                            psB[:, 0:512])
                                        nc.gpsimd.tensor_copy(
                                            lt[:, lc + 1536 : lc + 2048],
                                            psB[:, 512:1024])
                                        nc.gpsimd.tensor_copy(
                                            lt[:, lc + 2048 : lc + 2176],
                                            csl)
                                    else:
                                        nc.vector.tensor_copy(
                                            lt[:, lc : lc + 1024], psA[:]
                                        )
                                        nc.scalar.copy(
                                            lt[:, lc + 1024 : lc + 2048],
                                            psB[:]
                                        )
                                        if alt_c and (t % 2 == 1):
                                            nc.vector.tensor_copy(
                                                lt[:, lc + 2048 : lc + 2176],
                                                csl)
                                        else:
                                            nc.scalar.copy(
                                                lt[:, lc + 2048 : lc + 2176],
                                                csl)
                            # out[g, 128*(t0+b)+r, j] = lt[r, b*W + 127-r + j]
                            if nb == 1:
                                src_ap = [[W - 1, 128], [1, S]]
                            else:
                                src_ap = [[W, nb], [nb * W - 1, 128], [1, S]]
                            src = AP(
                                tensor=lt.tensor,
                                offset=lt.offset + 127,
                                ap=src_ap,
                            )
                            st_eng = (
                                nc.scalar
                                if dual_ring and (t0 % 2 == 1)
                                else nc.sync
                            )
                            if stores:
                                st_eng.dma_start(
                                    out[g, 128 * t0 : 128 * (t0 + nb), :], src
                                )
    if dedupe_lw:
        dedupe_ldweights(nc, mode=lw_mode)
    if dedupe_w:
        dedupe_waits(nc)
    nc.compile()
    return nc


def dedupe_ldweights(nc, mode="sig"):
    """Remove back-to-back InstLdweights that reload the identical
    stationary (the legalizer emits one per matmul; all 5 chunk matmuls
    of a row block share one stationary).  The PE queue is in-order, so
    the weights stay resident across the following matmuls.  Sem waits /
    updates of a deleted load are transferred to its paired matmul
    (which executes in the same queue slot) so semaphore thresholds are
    preserved.  Runs pre-compile: generate_event_semaphores afterwards
    re-normalizes multi-wait instructions.

    mode="all": strip every ldweights after the first per slot (WRONG
    results -- timing diagnostic only, isolates ldweights cost).
    """
    removed = 0
    last_sig = {}
    pending_sync = None
    for blk in nc.m.functions[0].blocks:
        if blk.IsLoopEntry or blk.IsPredicated:
            last_sig = {}
        insts = list(blk.instructions)
        new_insts = []
        for inst in insts:
            if isinstance(inst, mybir.InstLdweights):
                ap = inst.ins[0]
                # PE row-slot: folded pairs live at partitions 0-63 /
                # 64-127 (tile_position row 0 / 64), whose stationaries
                # are independent in the PE array.
                slot = ap.offset // (64 * S)
                sig = repr(ap)
                if (slot in last_sig) if mode == "all" else (
                        sig == last_sig.get(slot)):
                    si = inst.sync_info
                    if si is not None and (len(si.on_wait) or len(si.on_update)):
                        if pending_sync is None:
                            pending_sync = ([], [])
                        pending_sync[0].extend(si.on_wait)
                        pending_sync[1].extend(si.on_update)
                    removed += 1
                    continue
                last_sig[slot] = sig
            elif isinstance(inst, mybir.InstMatmult):
                if pending_sync is not None:
                    si = inst.sync_info
                    if si is None:
                        inst.sync_info = mybir.SyncInfo(
                            on_wait=pending_sync[0], on_update=pending_sync[1]
                        )
                    else:
                        si.on_wait = list(si.on_wait) + pending_sync[0]
                        si.on_update = list(si.on_update) + pending_sync[1]
                    pending_sync = None
            new_insts.append(inst)
        if len(new_insts) != len(insts):
            blk.instructions = new_insts
    assert pending_sync is None, "dangling sync from deleted ldweights"
    return removed


_WAIT_ENGINES = ("EngineType.PE", "EngineType.DVE",
                 "EngineType.Activation", "EngineType.SP")


def dedupe_waits(nc):
    """Drop semaphore waits that are subsumed by an earlier wait in the
    same engine's in-order queue (same sem, earlier threshold >= later,
    'sem-ge-imm' mode only — these are monotonically-incrementing
    completion counters).  The later instruction can then only issue
    earlier in wall-clock, never before its guaranteed dependencies.
    Fewer waits also means fewer InstEventSemaphore helpers after
    generate_event_semaphores (each costs SEQ time on the engine).
    Tracking resets at loop entries, barriers/drains/ISA ops, and on any
    non-increment update to a sem.
    """
    seen = {e: {} for e in _WAIT_ENGINES}
    removed = 0

    def clear_all():
        for e in _WAIT_ENGINES:
            seen[e].clear()

    for blk in nc.m.functions[0].blocks:
        if blk.IsLoopEntry or blk.IsPredicated:
            clear_all()
        for inst in blk.instructions:
            if isinstance(inst, (mybir.InstDrain, mybir.InstISA,
                                 mybir.InstAllEngineBarrier)):
                clear_all()
                continue
            eng = str(getattr(inst, "engine", ""))
            si = inst.sync_info
            if si is not None:
                for upd in si.on_update:
                    # non-increment update (set/clear) invalidates history
                    mode = getattr(upd, "update_mode", "")
                    if "inc" not in str(mode):
                        for e in _WAIT_ENGINES:
                            seen[e].pop(getattr(upd, "id", None), None)
            if eng not in _WAIT_ENGINES or si is None or not si.on_wait:
                continue
            tbl = seen[eng]
            keep = []
            for wt in si.on_wait:
                if wt.wait_mode == "sem-ge-imm" and wt.wait_reg is None:
                    prev = tbl.get(wt.id)
                    if prev is not None and prev >= wt.wait_value:
                        removed += 1
                        continue
                    tbl[wt.id] = max(prev or 0, wt.wait_value)
                keep.append(wt)
            if len(keep) != len(si.on_wait):
                si.on_wait = keep
    return removed


_NC_CACHE = {}

NB_PER_DMA = 1
LP_BUFS = 6

# Best-known configuration (see opt.py sweeps):
#  - eshare: one folded E load serves all 4 pairs (pairs h-major)
#  - pair_interleave: consecutive blocks alternate the two folded pairs,
#    whose stationaries sit in PE row-slots 0/64 -- the PE runs the two
#    64-contraction matmuls concurrently (~2.3x PE throughput)
#  - c512 evacuation with per-block alternating C-chunk engine
#  - fine_loads: q on SP ring / E on SWDGE in parallel, small-first
BEST_KW = dict(eshare=True, pair_interleave=True, scheme="c512",
               dve_chunks="alt", fine_loads=True, lp_bufs=6)


def _np_dtype(mm_dtype):
    return np.float16 if mm_dtype == mybir.dt.float16 else np.float32


def _get_nc(mm_dtype=MM_DTYPE, nb=None, lp_bufs=None, **kw):
    nb = NB_PER_DMA if nb is None else nb
    lp_bufs = LP_BUFS if lp_bufs is None else lp_bufs
    key = (str(mm_dtype), nb, lp_bufs, tuple(sorted(kw.items())))
    if key not in _NC_CACHE:
        _NC_CACHE[key] = build_nc(mm_dtype, nb=nb, lp_bufs=lp_bufs, **kw)
    return _NC_CACHE[key]


def make_in_maps(query, rel_emb, mm_dtype=MM_DTYPE, eshare=False):
    npdt = _np_dtype(mm_dtype)
    query = np.asarray(query, dtype=np.float32).astype(npdt)
    rel_emb = np.asarray(rel_emb, dtype=np.float32).astype(npdt)
    # [B,H,S,D] -> [32, D, S], pair p = n*16 + h
    qTt = np.ascontiguousarray(
        query.reshape(B * H, S, D).transpose(0, 2, 1)
    )
    in_maps = []
    for k in range(N_CORES):
        if eshare:
            # core k owns heads (2k, 2k+1) for BOTH batch entries; pair
            # order (n0,h0),(n0,h1),(n1,h0),(n1,h1) so one folded E load
            # [h0;h1] serves all four pairs.
            h0 = 2 * k
            pairs = [h0, h0 + 1, H + h0, H + h0 + 1]
            in_maps.append(
                {
                    "qT": np.ascontiguousarray(qTt[pairs]),
                    "E": np.ascontiguousarray(rel_emb[h0 : h0 + 2]),
                }
            )
        else:
            h0 = 4 * (k % 4)
            in_maps.append(
                {
                    "qT": qTt[4 * k : 4 * k + 4],
                    "E": np.ascontiguousarray(rel_emb[h0 : h0 + 4]),
                }
            )
    return in_maps


def run_sharded(query, rel_emb, trace=False, mm_dtype=MM_DTYPE, **kw):
    nc = _get_nc(mm_dtype, **kw)
    in_maps = make_in_maps(query, rel_emb, mm_dtype,
                           eshare=kw.get("eshare", False))
    last_exc = None
    for attempt in range(3):
        if attempt:
            # transient device errors (e.g. NRT_EXEC_UNIT_UNRECOVERABLE)
            # have been observed to clear after a short cooldown
            import time

            time.sleep(20 * attempt)
        try:
            res = run_bass_kernel_spmd(
                nc, in_maps, list(range(N_CORES)), trace=trace
            )
            break
        except Exception as exc:  # noqa: BLE001 - retry transient device faults
            last_exc = exc
    else:
        raise last_exc
    full = np.empty((B * H, S, S), dtype=np.float32)
    for k in range(N_CORES):
        if kw.get("eshare", False):
            h0 = 2 * k
            pairs = [h0, h0 + 1, H + h0, H + h0 + 1]
            full[pairs] = res.results[k]["out"]  # upcasts if fp16
        else:
            full[4 * k : 4 * k + 4] = res.results[k]["out"]
    return full.reshape(B, H, S, S), res


def kernel(query, rel_emb, sequence_length=None):
    out, _ = run_sharded(query, rel_emb, trace=False, **BEST_KW)
    return out


# ---------------------------------------------------------------------------
# Timing harness (dev only): re-runnable sharded executable without donation,
# pipelined dispatch, null-kernel baseline subtraction.
# ---------------------------------------------------------------------------


def _prepare_exec(nc, in_maps, chain=1):
    import jax
    from jax.experimental.shard_map import shard_map
    from jax.sharding import Mesh, NamedSharding, PartitionSpec

    from concourse import bass2jax, mybir as mb

    bass2jax.install_neuronx_cc_hook()
    n_cores = len(in_maps)

    in_names, out_names, out_avals, zero_outs = [], [], [], []
    for alloc in nc.m.functions[0].allocations:
        if not isinstance(alloc, mb.MemoryLocationSet):
            continue
        name = alloc.memorylocations[0].name
        if alloc.kind == "ExternalInput":
            in_names.append(name)
        elif alloc.kind == "ExternalOutput":
            out_names.append(name)
            shape = tuple(alloc.tensor_shape)
            dtype = mb.dt.np(alloc.dtype)
            out_avals.append(jax.core.ShapedArray(shape, dtype))
            zero_outs.append(np.zeros(shape, dtype))
    partition_name = (
        nc.partition_id_tensor.name if nc.partition_id_tensor else None
    )
    if partition_name is not None and partition_name in in_names:
        in_names.remove(partition_name)
    n_params = len(in_names)
    in_names = in_names + out_names
    if partition_name is not None:
        in_names.append(partition_name)

    def _body(*args):
        operands = list(args)
        if partition_name is not None:
            operands.append(bass2jax.partition_id_tensor())
        for _ in range(chain):
            outs = bass2jax._bass_exec_p.bind(
                *operands,
                out_avals=tuple(out_avals),
                in_names=tuple(in_names),
                out_names=tuple(out_names),
                lowering_input_output_aliases=(),
                sim_require_finite=True,
                sim_require_nnan=True,
                nc=nc,
            )
        return tuple(outs)

    devices = jax.devices()[:n_cores]
    mesh = Mesh(np.asarray(devices), ("core",))
    spec = PartitionSpec("core")
    sharded = jax.jit(
        shard_map(
            _body,
            mesh=mesh,
            in_specs=(spec,) * (n_params + len(out_names)),
            out_specs=(spec,) * len(out_names),
            check_rep=False,
        ),
        keep_unused=True,
    )
    sh = NamedSharding(mesh, spec)
    per_core = [[np.asarray(m[name]) for name in in_names[:n_params]]
                for m in in_maps]
    args = [
        jax.device_put(
            np.concatenate([per_core[c][i] for c in range(n_cores)], axis=0), sh
        )
        for i in range(n_params)
    ]
    args += [
        jax.device_put(
            np.zeros((n_cores * z.shape[0], *z.shape[1:]), z.dtype), sh
        )
        for z in zero_outs
    ]
    return sharded, args


def build_null_nc(mm_dtype=MM_DTYPE, out_dtype=OUT_DTYPE):
    """Same I/O signature, near-zero work: for dispatch-overhead baseline."""
    nc = bacc.Bacc("TRN2", target_bir_lowering=False, debug=False)
    qT = nc.declare_dram_parameter("qT", [G, D, S], mm_dtype, isOutput=False)
    nc.declare_dram_parameter("E", [G, D, J], mm_dtype, isOutput=False)
    out = nc.declare_dram_parameter("out", [G, S, S], out_dtype, isOutput=True)
    with tile.TileContext(nc) as tc:
        with tc.tile_pool(name="p", bufs=1) as p:
            t = p.tile([64, 128], mm_dtype, name="t")
            t2 = p.tile([64, 128], out_dtype, name="t2")
            nc.sync.dma_start(t[:], qT[0, :, :128])
            nc.vector.tensor_copy(t2[:], t[:])
            nc.sync.dma_start(out[0, :64, :128], t2[:])
    nc.compile()
    return nc


def _time_callable(f, args, iters, reps=3):
    import time as _t

    import jax

    out = f(*args)
    jax.block_until_ready(out)
    best = float("inf")
    for _ in range(reps):
        t0 = _t.perf_counter()
        outs = [f(*args) for _ in range(iters)]
        jax.block_until_ready(outs)
        t1 = _t.perf_counter()
        best = min(best, (t1 - t0) / iters)
        del outs
    return best


def model_time_ns(mm_dtype=MM_DTYPE, **kw):
    """Instruction-level cost-model (TimelineSim) estimate for one core."""
    from concourse.timeline_sim import TimelineSim

    return TimelineSim(_get_nc(mm_dtype, **kw), trace=False).simulate()


def time_kernel(query, rel_emb, iters=6, mm_dtype=MM_DTYPE, rounds=4, **kw):
    """Differential wall-clock: alternate (kernel, null-kernel with same I/O)
    pipelined batches; report median of per-round differences.  The axon
    dispatch overhead (~3 ms/call, noisy) mostly cancels; the cost-model
    estimate is typically the more trustworthy number."""
    in_maps = make_in_maps(query, rel_emb, mm_dtype)
    f, args = _prepare_exec(_get_nc(mm_dtype, **kw), in_maps)
    f0, args0 = _prepare_exec(build_null_nc(mm_dtype), in_maps)
    tks, tns = [], []
    for _ in range(rounds):
        tks.append(_time_callable(f, args, iters, reps=1))
        tns.append(_time_callable(f0, args0, iters, reps=1))
    best = min(tks) - min(tns)
    print(f"  min kernel={min(tks)*1e6:.0f}us  min null={min(tns)*1e6:.0f}us  "
          f"diff-of-mins={best*1e6:.0f}us")
    return best * 1e9

